# revision 9
# baseline (speedup 1.0000x reference)
"""GCN encoder (2x GraphConv + per-graph mean pool) on 8 Trainium2 NeuronCores.

Strategy:
  - Shard nodes by graph boundaries (graph_ids is sorted) -> pooling is
    core-local. Each core owns a contiguous node range, padded into a
    fixed SLOT-row segment of a "gathered table" of NCORES*SLOT rows.
  - Per layer: h_tilde = norm_src * h computed for own rows, AllGather to
    give every core the full table, then per-core message aggregation:
      dma_gather of h_tilde[src] rows (512B rows = full-rate descriptors)
      + onehot matmuls on the TensorEngine accumulating
        agg[f, d] += E_chunk.T @ onehot(dst_local)  into PSUM,
      then agg @ W, per-partition norm_dst/bias/relu epilogue.
  - dma_gather indices are int16 (<32768), so gathers read from the table
    tensor directly (rows < SPLIT) or from a copied "hi" tensor
    (rows >= SPLIT), with SPLIT on a core-slot boundary.
  - SPMD: one program on 8 cores; per-(tile, lo/hi) chunk counts are
    equalized across cores at preprocessing time (padding edges with
    sentinel dst -> zero onehot column).
"""

import sys
import numpy as np

sys.path.insert(0, "/opt/trn_rl_repo")

D = 128
P = 128  # partitions / tile rows


class Cfg:
    def __init__(self, n_nodes, n_edges, n_graphs, ncores=8, slot=6400,
                 split_ranks=5, call_chunks=8):
        self.N = n_nodes
        self.E = n_edges
        self.G = n_graphs
        self.C = ncores
        self.SLOT = slot
        assert slot % P == 0
        self.TILES = slot // P
        self.TROWS = ncores * slot
        self.SPLIT = split_ranks * slot  # lo/hi table split (core boundary)
        assert 0 < self.SPLIT <= 32768
        assert self.TROWS - self.SPLIT <= 32768
        self.HIROWS = self.TROWS - self.SPLIT
        self.CALL = call_chunks  # chunks (of 128 idxs) per dma_gather call


def _wrap_idx(stream):
    """[L] -> [128, L/16] int16: idx i at [i%16, i//16], replicated x8."""
    L = len(stream)
    assert L % 16 == 0
    w = stream.reshape(L // 16, 16).T.astype(np.int16)
    return np.ascontiguousarray(np.tile(w, (8, 1)))


def preprocess(node_feats, W1, b1, W2, b2, src, dst, graph_ids, cfg):
    """All-integer index preprocessing + per-core input arrays."""
    N, G, C, SLOT = cfg.N, cfg.G, cfg.C, cfg.SLOT
    src = np.asarray(src, dtype=np.int64)
    dst = np.asarray(dst, dtype=np.int64)
    gid = np.asarray(graph_ids, dtype=np.int64)
    x = np.asarray(node_feats, dtype=np.float32)

    sizes = np.bincount(gid, minlength=G)
    gstart = np.concatenate([[0], np.cumsum(sizes)])  # [G+1] node offsets

    # assign graphs to cores: cut at graph boundary nearest to c*N/C
    cuts = [0]
    for c in range(1, C):
        ideal = c * N / C
        g = int(np.argmin(np.abs(gstart - ideal)))
        cuts.append(g)
    cuts.append(G)
    core_g0 = np.array(cuts[:-1])
    core_g1 = np.array(cuts[1:])
    core_n0 = gstart[core_g0]
    core_n1 = gstart[core_g1]
    n_own = core_n1 - core_n0
    assert (n_own <= SLOT).all(), n_own
    assert ((core_g1 - core_g0) <= P).all()

    # global node -> (core, gathered-table row)
    core_of = np.searchsorted(core_n1, np.arange(N), side="right")
    R = SLOT * core_of + (np.arange(N) - core_n0[core_of])

    deg_out = np.maximum(np.bincount(src, minlength=N), 1).astype(np.float32)
    deg_in = np.maximum(np.bincount(dst, minlength=N), 1).astype(np.float32)

    Rsrc_all = R[src]
    e_core = core_of[dst]

    # per (core, tile, lo/hi) edge lists; then equalize chunk counts
    per_core = []
    for c in range(C):
        m = e_core == c
        es, ed = Rsrc_all[m], dst[m] - core_n0[c]
        order = np.argsort(ed, kind="stable")
        es, ed = es[order], ed[order]
        t_of = ed // P
        lo = es < cfg.SPLIT
        tiles = []
        for t in range(cfg.TILES):
            tm = t_of == t
            tiles.append((
                (es[tm & lo], (ed[tm & lo] % P)),
                (es[tm & ~lo] - cfg.SPLIT, (ed[tm & ~lo] % P)),
            ))
        per_core.append(tiles)

    K_lo = np.zeros(cfg.TILES, dtype=np.int64)
    K_hi = np.zeros(cfg.TILES, dtype=np.int64)
    for t in range(cfg.TILES):
        K_lo[t] = max(-(-len(per_core[c][t][0][0]) // P) for c in range(C))
        K_hi[t] = max(-(-len(per_core[c][t][1][0]) // P) for c in range(C))
        if K_lo[t] + K_hi[t] == 0:
            K_lo[t] = 1  # guarantee >=1 chunk so PSUM is always initialized

    def build_stream(c, which, K):
        idxs, dlocs = [], []
        for t in range(cfg.TILES):
            e_idx, e_dl = per_core[c][t][which]
            pad = K[t] * P - len(e_idx)
            idxs.append(np.concatenate([e_idx, np.zeros(pad, np.int64)]))
            dlocs.append(np.concatenate([e_dl, np.full(pad, -1, np.int64)]))
        return np.concatenate(idxs), np.concatenate(dlocs)

    in_maps = []
    for c in range(C):
        il, dl_l = build_stream(c, 0, K_lo)
        ih, dl_h = build_stream(c, 1, K_hi)
        n0, n1 = core_n0[c], core_n1[c]
        no = n1 - n0

        x_own = np.zeros((SLOT, D), np.float32)
        x_own[:no] = x[n0:n1]

        def slotf(vals, fill):
            a = np.full(SLOT, fill, np.float32)
            a[:no] = vals
            return a.reshape(cfg.TILES, P).T.copy()  # [128, TILES]

        gl_own = slotf(gid[n0:n1] - core_g0[c], -1.0)
        counts = np.ones((P, 1), np.float32)
        ng = core_g1[c] - core_g0[c]
        counts[:ng, 0] = np.maximum(sizes[core_g0[c]:core_g1[c]], 1)

        in_maps.append({
            "x_own": x_own,
            "W1": np.asarray(W1, np.float32), "W2": np.asarray(W2, np.float32),
            "b1": np.asarray(b1, np.float32), "b2": np.asarray(b2, np.float32),
            "deg_out": slotf(deg_out[n0:n1], 1.0),
            "deg_in": slotf(deg_in[n0:n1], 1.0),
            "gl": gl_own,
            "counts": counts,
            "idx_lo": _wrap_idx(il), "idx_hi": _wrap_idx(ih),
            "dl_lo": dl_l.reshape(-1, P).T.astype(np.float32).copy(),
            "dl_hi": dl_h.reshape(-1, P).T.astype(np.float32).copy(),
        })

    meta = dict(K_lo=K_lo, K_hi=K_hi, core_g0=core_g0, core_g1=core_g1,
                L_lo=len(in_maps[0]["idx_lo"][0]) * 16,
                L_hi=len(in_maps[0]["idx_hi"][0]) * 16)
    return in_maps, meta


def build_program(cfg, meta):
    import concourse.bass as bass
    import concourse.bacc as bacc
    import concourse.tile as tile
    import concourse.mybir as mybir
    from concourse import library_config

    dt = mybir.dt
    Alu = mybir.AluOpType
    Act = mybir.ActivationFunctionType
    K_lo, K_hi = meta["K_lo"], meta["K_hi"]
    C_lo, C_hi = int(K_lo.sum()), int(K_hi.sum())
    TILES = cfg.TILES

    nc = bacc.Bacc("TRN2", target_bir_lowering=False, debug=False,
                   num_devices=cfg.C)

    t_x = nc.dram_tensor("x_own", [cfg.SLOT, D], dt.float32, kind="ExternalInput")
    t_W = [nc.dram_tensor(f"W{l+1}", [D, D], dt.float32, kind="ExternalInput")
           for l in range(2)]
    t_b = [nc.dram_tensor(f"b{l+1}", [D], dt.float32, kind="ExternalInput")
           for l in range(2)]
    t_dego = nc.dram_tensor("deg_out", [P, TILES], dt.float32, kind="ExternalInput")
    t_degi = nc.dram_tensor("deg_in", [P, TILES], dt.float32, kind="ExternalInput")
    t_gl = nc.dram_tensor("gl", [P, TILES], dt.float32, kind="ExternalInput")
    t_counts = nc.dram_tensor("counts", [P, 1], dt.float32, kind="ExternalInput")
    t_idx = {"lo": nc.dram_tensor("idx_lo", [P, meta["L_lo"] // 16], dt.int16,
                                  kind="ExternalInput"),
             "hi": nc.dram_tensor("idx_hi", [P, meta["L_hi"] // 16], dt.int16,
                                  kind="ExternalInput")}
    t_dl = {"lo": nc.dram_tensor("dl_lo", [P, C_lo], dt.float32,
                                 kind="ExternalInput"),
            "hi": nc.dram_tensor("dl_hi", [P, C_hi], dt.float32,
                                 kind="ExternalInput")}
    t_out = nc.dram_tensor("pool_out", [P, D], dt.float32, kind="ExternalOutput")

    nchunks = {"lo": C_lo, "hi": C_hi}
    ncalls = {s: -(-nchunks[s] // cfg.CALL) for s in ("lo", "hi")}

    with tile.TileContext(nc) as tc:
        nc.gpsimd.load_library(library_config.mlp)
        with (
            tc.tile_pool(name="const", bufs=1) as constp,
            tc.tile_pool(name="xin", bufs=3) as xinp,
            tc.tile_pool(name="tabw", bufs=3) as tabwp,
            tc.tile_pool(name="glo", bufs=4) as gpool_lo,
            tc.tile_pool(name="ghi", bufs=4) as gpool_hi,
            tc.tile_pool(name="ohlo", bufs=4) as ohpool_lo,
            tc.tile_pool(name="ohhi", bufs=4) as ohpool_hi,
            tc.tile_pool(name="epi", bufs=3) as epip,
            tc.tile_pool(name="pagg", bufs=2, space="PSUM") as paggp,
            tc.tile_pool(name="p2", bufs=2, space="PSUM") as p2p,
            tc.tile_pool(name="ppool", bufs=1, space="PSUM") as ppoolp,
            tc.tile_pool(name="dram", bufs=1, space="DRAM") as dramp,
        ):
            # ---- constants
            W_sb, b_bc = [], []
            for l in range(2):
                w = constp.tile([D, D], dt.float32, name=f"wsb{l}")
                nc.sync.dma_start(w[:], t_W[l][:])
                W_sb.append(w)
                bb = constp.tile([P, D], dt.float32, name=f"bbc{l}")
                nc.sync.dma_start(bb[:], bass.AP(t_b[l].ap().tensor, 0,
                                                 [[0, P], [1, D]]))
                b_bc.append(bb)
            iota1 = constp.tile([P, P], dt.float32, tag="iota1")
            nc.gpsimd.iota(iota1[:], pattern=[[1, P]], base=0,
                           channel_multiplier=0,
                           allow_small_or_imprecise_dtypes=True)
            iotaC = constp.tile([P, cfg.CALL, P], dt.float32, tag="iotaC")
            nc.gpsimd.iota(iotaC[:], pattern=[[0, cfg.CALL], [1, P]], base=0,
                           channel_multiplier=0,
                           allow_small_or_imprecise_dtypes=True)

            def load_norm(tensor, tag):
                deg = constp.tile([P, TILES], dt.float32, name=f"deg{tag}")
                nc.sync.dma_start(deg[:], tensor[:])
                rec = constp.tile([P, TILES], dt.float32, name=f"rec{tag}")
                nc.vector.reciprocal(rec[:], deg[:])
                nrm = constp.tile([P, TILES], dt.float32, name=f"nrm{tag}")
                nc.scalar.activation(nrm[:], rec[:], Act.Sqrt)
                return nrm

            norm_src = load_norm(t_dego, "s")
            norm_dst = load_norm(t_degi, "d")

            gl_sb = constp.tile([P, TILES], dt.float32, tag="gl")
            nc.sync.dma_start(gl_sb[:], t_gl[:])
            counts_sb = constp.tile([P, 1], dt.float32, tag="cnt")
            nc.sync.dma_start(counts_sb[:], t_counts[:])

            idx_sb, dl_sb = {}, {}
            for s in ("lo", "hi"):
                it = constp.tile(list(t_idx[s].shape), dt.int16, name=f"idxsb{s}")
                nc.sync.dma_start(it[:], t_idx[s][:])
                idx_sb[s] = it
                dlt = constp.tile(list(t_dl[s].shape), dt.float32, name=f"dlsb{s}")
                nc.sync.dma_start(dlt[:], t_dl[s][:])
                dl_sb[s] = dlt

            # ---- DRAM interchange buffers.
            # NB: real dram_tensors, NOT pool tiles — dma_gather's DynamicAP
            # needs the source at offset 0 of an actual tensor allocation.
            ag_in = [nc.dram_tensor(f"agin{l}", [cfg.SLOT, D], dt.float32,
                                    kind="Internal").ap() for l in range(2)]
            T_all = [nc.dram_tensor(f"tall{l}", [cfg.TROWS, D], dt.float32,
                                    kind="Internal", addr_space="Shared").ap()
                     for l in range(2)]
            T_hi = [nc.dram_tensor(f"thi{l}", [cfg.HIROWS, D], dt.float32,
                                   kind="Internal").ap() for l in range(2)]

            # ---- phase 1: table1 = norm_src * x  (own rows)
            for t in range(TILES):
                xt = xinp.tile([P, D], dt.float32, tag="xt")
                nc.sync.dma_start(xt[:], t_x[t * P:(t + 1) * P, :])
                tb = tabwp.tile([P, D], dt.float32, tag="tb")
                nc.vector.tensor_scalar(out=tb[:], in0=xt[:],
                                        scalar1=norm_src[:, t:t + 1],
                                        scalar2=None, op0=Alu.mult)
                nc.sync.dma_start(ag_in[0][t * P:(t + 1) * P, :], tb[:])

            pool_ps = ppoolp.tile([P, D], dt.float32, tag="pool")

            stage = getattr(cfg, "stage", 99)

            def dump(src_ap):
                """debug: bounce 128 DRAM rows to out and stop."""
                z = constp.tile([P, D], dt.float32, tag="dbg")
                nc.sync.dma_start(z[:], src_ap)
                nc.sync.dma_start(t_out[:], z[:])

            # per-stream schedule: chunk j of stream -> (call j//CALL, slot j%CALL)
            for l in range(2):
                if stage <= 1:
                    dump(ag_in[0][0:P, :])
                    break
                nc.gpsimd.collective_compute(
                    "AllGather", Alu.bypass,
                    ins=[ag_in[l]], outs=[T_all[l]],
                    replica_groups=[list(range(cfg.C))],
                )
                if stage <= 2:
                    dump(T_all[0][0:P, :])
                    break
                nc.sync.dma_start(T_hi[l][:], T_all[l][cfg.SPLIT:, :])
                if stage <= 3:
                    dump(T_hi[0][0:P, :])
                    break

                call_tiles = {"lo": {}, "hi": {}}

                def emit_call(s, k, l=l, call_tiles=call_tiles):
                    n = min(cfg.CALL, nchunks[s] - k * cfg.CALL)
                    gp = gpool_lo if s == "lo" else gpool_hi
                    op = ohpool_lo if s == "lo" else ohpool_hi
                    g = gp.tile([P, cfg.CALL, D], dt.float32, name=f"g{s}{l}_{k}", tag=f"g{s}")
                    src_t = T_all[l] if s == "lo" else T_hi[l]
                    nidx = n * P
                    nc.gpsimd.dma_gather(
                        g[:, :n, :], src_t[:],
                        idx_sb[s][:, k * cfg.CALL * 8:(k * cfg.CALL * 8) + nidx // 16],
                        nidx, nidx, D)
                    oh = op.tile([P, cfg.CALL, P], dt.float32, name=f"oh{s}{l}_{k}", tag=f"oh{s}")
                    dslice = dl_sb[s][:, k * cfg.CALL:k * cfg.CALL + n]
                    nc.vector.tensor_tensor(
                        out=oh[:, :n, :], in0=iotaC[:, :n, :],
                        in1=dslice.unsqueeze(2).broadcast_to([P, n, P]),
                        op=Alu.is_equal)
                    call_tiles[s][k] = (g, oh)

                pos = {"lo": 0, "hi": 0}
                for t in range(TILES):
                    nk = int(K_lo[t] + K_hi[t])
                    agg = paggp.tile([P, P], dt.float32, tag="agg")
                    ci = 0
                    for s, K in (("lo", K_lo), ("hi", K_hi)):
                        for _ in range(int(K[t])):
                            j = pos[s]
                            k, slot = j // cfg.CALL, j % cfg.CALL
                            if k not in call_tiles[s]:
                                emit_call(s, k)
                            g, oh = call_tiles[s][k]
                            nc.tensor.matmul(agg[:], lhsT=g[:, slot, :],
                                             rhs=oh[:, slot, :],
                                             start=(ci == 0), stop=(ci == nk - 1))
                            pos[s] = j + 1
                            ci += 1
                    # epilogue: h = relu(norm_dst * (agg.T @ W) + b)
                    aggs = epip.tile([P, P], dt.float32, tag="aggs")
                    nc.vector.tensor_copy(aggs[:], agg[:])
                    ps2 = p2p.tile([P, D], dt.float32, tag="ps2")
                    nc.tensor.matmul(ps2[:], lhsT=aggs[:], rhs=W_sb[l][:],
                                     start=True, stop=True)
                    s1 = epip.tile([P, D], dt.float32, tag="s1")
                    nc.vector.tensor_scalar(out=s1[:], in0=ps2[:],
                                            scalar1=norm_dst[:, t:t + 1],
                                            scalar2=None, op0=Alu.mult)
                    s2 = epip.tile([P, D], dt.float32, tag="s2")
                    nc.vector.tensor_tensor(out=s2[:], in0=s1[:], in1=b_bc[l][:],
                                            op=Alu.add)
                    if l == 0:
                        tb2 = tabwp.tile([P, D], dt.float32, tag="tb2")
                        nc.vector.tensor_scalar(out=tb2[:], in0=s2[:],
                                                scalar1=0.0,
                                                scalar2=norm_src[:, t:t + 1],
                                                op0=Alu.max, op1=Alu.mult)
                        nc.sync.dma_start(ag_in[1][t * P:(t + 1) * P, :], tb2[:])
                    else:
                        h3 = epip.tile([P, D], dt.float32, tag="h3")
                        nc.vector.tensor_scalar(out=h3[:], in0=s2[:],
                                                scalar1=0.0, scalar2=None,
                                                op0=Alu.max)
                        ohp = epip.tile([P, P], dt.float32, tag="ohp")
                        nc.vector.tensor_scalar(out=ohp[:], in0=iota1[:],
                                                scalar1=gl_sb[:, t:t + 1],
                                                scalar2=None, op0=Alu.is_equal)
                        nc.tensor.matmul(pool_ps[:], lhsT=ohp[:], rhs=h3[:],
                                         start=(t == 0), stop=(t == TILES - 1),
                                         skip_group_check=True)
                if stage <= 4 and l == 0:
                    dump(ag_in[1][0:P, :])
                    break

            if stage >= 5:
                # ---- pool epilogue: mean = pool / counts
                rc = constp.tile([P, 1], dt.float32, tag="rc")
                nc.vector.reciprocal(rc[:], counts_sb[:])
                po = constp.tile([P, D], dt.float32, tag="po")
                nc.vector.tensor_scalar(out=po[:], in0=pool_ps[:], scalar1=rc[:],
                                        scalar2=None, op0=Alu.mult)
                nc.sync.dma_start(t_out[:], po[:])

    nc.compile()
    return nc


_cache = {}


def kernel(node_feats, W1, b1, W2, b2, src, dst, graph_ids):
    from concourse.bass_utils import run_bass_kernel_spmd

    assert node_feats.shape == (50000, 128), node_feats.shape
    cfg = Cfg(50000, len(np.asarray(src)), 500)

    key = (node_feats.shape, hash(np.asarray(src).tobytes()),
           hash(np.asarray(dst).tobytes()),
           hash(np.asarray(graph_ids).tobytes()))
    in_maps, meta = preprocess(node_feats, W1, b1, W2, b2, src, dst,
                               graph_ids, cfg)
    if key in _cache:
        nc = _cache[key]
    else:
        nc = build_program(cfg, meta)
        _cache[key] = nc

    res = run_bass_kernel_spmd(nc, in_maps, core_ids=list(range(cfg.C)))

    out = np.zeros((cfg.G, D), np.float32)
    for c in range(cfg.C):
        g0, g1 = meta["core_g0"][c], meta["core_g1"][c]
        out[g0:g1] = res.results[c]["pool_out"][:g1 - g0]
    return out
